# revision 1
# baseline (speedup 1.0000x reference)
"""Trainium2 Bass kernel for greedy sequential independent-set sampling.

Reference semantics: sites visited in row-major order; site (r, c) is set to 1
iff u[s, r, c] < 0.5 and no already-set lattice neighbor. Because the visit
order is row-major, right/down neighbors are still 0 when a site is decided:

    x[r, c] = (u[r, c] < 0.5) & ~x[r-1, c] & ~x[r, c-1]

One DVE tensor_tensor_scan per lattice row computes the whole thing:

    state' = (nb[c] - state) is_gt x_prev[c]      (op0=subtract, op1=is_gt)

where nb = sign(0.5 - u) in {-1,0,+1} (ScalarE) and state carries x[r, c-1]:
nb - x_left = 1 iff bernoulli hit AND left free; > x_up iff up free.

Sample axis is data-parallel: 65536 samples -> 8 cores x 8192 samples; per
core 64 groups of 128 samples (SBUF partition dim), 2 chunks of 32 groups.
Groups are packed side by side in the scan's free dim with one dummy column
(nb = -1) per group so the carried state resets to 0 at group boundaries.
The host pre-permutes u into a row-slab-major layout and post-permutes the
output so every DMA descriptor is >= 8 KiB contiguous per partition.
"""

import numpy as np

import concourse.bacc as bacc
import concourse.mybir as mybir
from concourse.tile import TileContext
from concourse.bass_utils import run_bass_kernel_spmd

N_CORES = 8
S_TOTAL = 65536
R = 32
C = 32
SITES = R * C  # 1024
P = 128  # SBUF partitions

SPC = S_TOTAL // N_CORES  # samples per core: 8192
G_TOTAL = SPC // P  # 64 groups of 128 samples

# Tunables
CHUNK_G = 32  # groups per chunk
SLAB_R = 8  # lattice rows per streamed-in u slab
OUT_R = 4  # lattice rows per staged output DMA (streams during the scan chain)
W = C + 1  # 33: one dummy col per group resets the scan carry

F32 = mybir.dt.float32
I8 = mybir.dt.int8
I32 = mybir.dt.int32


def build_nc(spc=SPC, chunk_g=CHUNK_G):
    """Build the per-core Bass program (SPMD: same program, different data)."""
    g_total = spc // P
    n_chunks = g_total // chunk_g
    n_slabs = R // SLAB_R
    slab_elems = chunk_g * SLAB_R * C  # free-dim elems of one u slab
    L = chunk_g * W  # one packed lattice row (with dummies)

    nc = bacc.Bacc("TRN2", target_bir_lowering=False, debug=False)
    # Host-permuted input: the top byte of each fp32 u value (u < 0.5 iff
    # byte3 < 0x3F for u in [0,1)), laid out [k][p][r][g][w] with a 0x7F
    # dummy byte at w=0 of each group (Sign turns it into the -1 that
    # resets the scan carry); any row range is one contiguous run.
    u = nc.declare_dram_parameter(
        "u", [n_chunks, P, R * chunk_g * W], I8, isOutput=False
    )
    # Output layout [k][p][r][g][c], row-contiguous per partition likewise.
    cfg = nc.declare_dram_parameter(
        "config", [n_chunks, P, R * chunk_g * C], I32, isOutput=True
    )

    with TileContext(nc) as tc:
        with (
            tc.tile_pool(name="const", bufs=1) as constp,
            tc.tile_pool(name="slab", bufs=2) as slabp,
            tc.tile_pool(name="nb", bufs=n_chunks) as nbp,
            tc.tile_pool(name="xall", bufs=2) as xallp,
            tc.tile_pool(name="stage", bufs=2) as stagep,
        ):
            thr = constp.tile([P, 1], F32, tag="thr")
            nc.gpsimd.memset(thr[:], 62.5)

            # Prefetch phase: DMA + Sign for ALL chunks first, so no chunk's
            # scans ever wait on a Sign queued behind another chunk's output
            # copies on the in-order ScalarE queue.
            nbs = []
            for k in range(n_chunks):
                # nb rows, packed like x rows: [r][g][w] with col 0 = dummy
                nb = nbp.tile([P, R * L], I8, tag="nb")
                # chunk 0 streams its first rows in small pieces so the
                # scan chain starts ~3 us in instead of waiting a full
                # 8-row Sign; later slabs use 8-row (~8 KiB) transfers.
                segs = [1, 1, 2, 4] + [SLAB_R] * ((R - 8) // SLAB_R) if k == 0 \
                    else [SLAB_R] * n_slabs
                a = 0
                for nr in segs:
                    slab = slabp.tile([P, SLAB_R * L], I8, tag="slab")
                    seg = slab[:, 0 : nr * L]
                    nc.sync.dma_start(
                        out=seg, in_=u[k][:, a * L : (a + nr) * L]
                    )
                    # nb = sign(62.5 - byte3) in {-1, +1}: +1 iff u < 0.5;
                    # the 0x7F dummy bytes come out as the -1 carry reset
                    nc.scalar.activation(
                        out=nb[:, a * L : (a + nr) * L],
                        in_=seg,
                        func=mybir.ActivationFunctionType.Sign,
                        bias=thr[:],
                        scale=-1.0,
                    )
                    a += nr
                nbs.append(nb)

            for k in range(n_chunks):
                nb = nbs[k]
                # x rows; slot 0 = virtual row -1 (zeros)
                x_all = xallp.tile([P, (R + 1) * L], I8, tag="xall")
                nc.vector.memset(x_all[:, 0:L], 0)

                x_rows = x_all[:, L:].rearrange(
                    "p (r g w) -> p r g w", g=chunk_g, w=W
                )
                def stage_out(a, nr):
                    # convert finished rows [a, a+nr) int8 -> int32, stream out
                    st = stagep.tile([P, OUT_R * chunk_g * C], I32, tag="stage")
                    seg = st[:, 0 : nr * chunk_g * C]
                    x_v = x_rows[:, a : a + nr, :, 1:W]
                    st_v = seg.rearrange("p (rr g c) -> p rr g c", g=chunk_g, c=C)
                    nc.scalar.activation(
                        out=st_v, in_=x_v, func=mybir.ActivationFunctionType.Copy
                    )
                    nc.sync.dma_start(
                        out=cfg[k][:, a * chunk_g * C : (a + nr) * chunk_g * C],
                        in_=seg,
                    )

                # Stage a finished row block only once the scan chain is LAG
                # rows past it: the copy's x_all reads must stay a full SBUF
                # bank (2 KiB) behind the next scan's write, or Tile's
                # bank-level WAR tracking serializes the scan chain.
                # The last rows go out in small pieces to shorten the tail.
                LAG = max(2, -(-2048 // L))
                blocks = [(a, OUT_R) for a in range(0, R - 4, OUT_R)]
                blocks += [(R - 4, 2), (R - 2, 1), (R - 1, 1)]
                bi = 0
                for r in range(R):
                    # state' = (nb - state) > x_up : the full site update
                    nc.vector.tensor_tensor_scan(
                        out=x_all[:, (r + 1) * L : (r + 2) * L],
                        data0=nb[:, r * L : (r + 1) * L],
                        data1=x_all[:, r * L : (r + 1) * L],
                        initial=0.0,
                        op0=mybir.AluOpType.subtract,
                        op1=mybir.AluOpType.is_gt,
                    )
                    while bi < len(blocks) and blocks[bi][0] + blocks[bi][1] + LAG <= r + 1:
                        stage_out(*blocks[bi])
                        bi += 1
                while bi < len(blocks):
                    stage_out(*blocks[bi])
                    bi += 1
    nc.compile()
    return nc


def host_permute_u(u_core, chunk_g=CHUNK_G):
    """[spc, 32, 32] f32 -> top-byte plane [n_chunks, n_slabs, P, ...] int8.

    For u in [0, 1), u < 0.5 iff the fp32 top byte (sign + exp[7:1]) is
    < 0x3F; only that byte is shipped to the device (4x less input DMA).
    """
    spc = u_core.shape[0]
    n_chunks = spc // P // chunk_g
    b3 = u_core.reshape(-1).view(np.uint8)[3::4]
    v = b3.reshape(n_chunks, chunk_g, P, R, C).transpose(0, 2, 3, 1, 4)
    out = np.full((n_chunks, P, R, chunk_g, W), 0x7F, np.uint8)
    out[..., 1:] = v
    return out.view(np.int8).reshape(n_chunks, P, R * chunk_g * W)


def host_unpermute_cfg(cfg_t, chunk_g=CHUNK_G):
    """[n_chunks, P, R*chunk_g*C] int32 -> [spc, 32, 32]."""
    n_chunks = cfg_t.shape[0]
    spc = n_chunks * chunk_g * P
    v = cfg_t.reshape(n_chunks, P, R, chunk_g, C)
    # sample s = k*chunk_g*P + g*P + p
    return np.ascontiguousarray(v.transpose(0, 3, 1, 2, 4)).reshape(spc, R, C)


_NC_CACHE = {}


def _get_nc():
    if "nc" not in _NC_CACHE:
        _NC_CACHE["nc"] = build_nc()
    return _NC_CACHE["nc"]


def kernel(u, n_rows=32, n_cols=32, **_):
    u = np.ascontiguousarray(np.asarray(u), dtype=np.float32)
    assert u.shape == (S_TOTAL, R, C), u.shape
    assert int(n_rows) == R and int(n_cols) == C

    nc = _get_nc()
    in_maps = [
        {"u": host_permute_u(u[i * SPC : (i + 1) * SPC])} for i in range(N_CORES)
    ]
    res = run_bass_kernel_spmd(nc, in_maps, list(range(N_CORES)))
    out = np.concatenate(
        [host_unpermute_cfg(res.results[i]["config"]) for i in range(N_CORES)],
        axis=0,
    )
    return out.astype(np.int32, copy=False).reshape(S_TOTAL, R, C)

